# revision 42
# baseline (speedup 1.0000x reference)
"""DiceEmbedding kernel for 8 Trainium2 NeuronCores.

Reference math (per element v of batch_val [262144]):
    theta    = ln(0.01 + |v|) / 85 * pi
    s, c     = sin(theta), cos(theta)
    polar    = [c, s*c, s^2*c, ..., s^8*c, s^10]           # [10]
    out      = (polar @ Q.T) @ W.T + b                     # [1024]

Key observation: out is a smooth 1-D function of L = ln(0.01+|v|) alone
(theta spans only [-0.17, 0.15] rad).  Host fits a QUARTIC in
z = (L - c)/r (z in [-1,1] for |v| <= 80) to the exact function
g(v) = polar @ (W@Q)^T + b per output column:  out ~= sum_k A_k z^k.
Fit residual < 1e-5 relative; total device error (bf16 powers, bf16 A,
int8 output) measures ~0.9% of absmax vs the 2% gate.

Device dataflow per core (data-parallel over N: 32768 elems per core):
  - batch slice arrives [128, 256] partition-major (x[p,t] = v[p*256+t])
  - ACT: |x| -> ln -> L (f32); DVE: z = (L-c)/r (bf16), z2 = z*z
  - DVE writes the 5-row basis [z, z2, z3, z4, ones] into P[128, 64*128]
    bf16 at cols st*128 + 32q + j (j<5; col 32q+4 is the memset ones row
    carrying A0; cols 5..31 untouched garbage, never contracted)
  - lhsT tiles via batched PE transposes: each packed [128,512] bf16
    PSUM tile holds FOUR [128,128] transposes (4 STs) in one bank, one
    DVE copy -> SBUF; batching 4-8 STs bounds the PE tiling-mode
    switches (128x128 transpose vs 32-row-tiled MMs drain the PE).
    (DMA-xbar transposes would free the PE entirely but hard-wedge the
    device next to the store DMAs — NRT_EXEC_UNIT_UNRECOVERABLE.)
  - per super-tile: 8 bf16 matmuls [K=5, N=512], tile_position=(32q,0),
    issued (q0h0, q1h0, q0h1, q1h1) so consecutive MMs alternate PE row
    groups (2-way concurrent streaming, hidden LDWEIGHTS) while each
    [128,1024] PSUM tile (2 banks; 3-buf pool + 2 transpose banks = all
    8 banks) fills its h0 then h1 chunk of one q
  - PSUM->SBUF f32->int8 casts at FD=1024, contiguous both sides (ring
    is in DRAM order; chunks (q,h0),(q,h1) are adjacent), split DVE:ACT
    ~ 10:11 (both engines read f32 PSUM at 1 elem/lane/cyc; FD=1024
    amortizes the ~120/172-cycle startup; strided dsts cost +~200cyc so
    contiguity matters)
  - per-ST 512 KiB contiguous DMA stores from an 8-ST int8 ring; host
    dequantizes (astype(f32) * SMAX/127)

Bottleneck: PSUM evacuation (every output crosses PSUM->SBUF on DVE or
ACT at 1 col/cycle) balanced against the PE stream (~290ns per 512-col
MM incl overheads).  Floor ~ 150 us busy + ~15 us framework pre/post-
amble; v1 (11-row basis, 512-col casts, per-pair PE transposes) ran
281 us.
"""

import numpy as np

D = 10
EMB = 1024
N_TOTAL = 262144
N_CORES = 8
N_PER_CORE = N_TOTAL // N_CORES          # 32768
TILES_PER_CORE = N_PER_CORE // 128       # 256
SUPER = 4                                # batch tiles per super-tile
N_SUPER = TILES_PER_CORE // SUPER        # 64
KDIM = 5                                 # quartic basis: z..z^4 + ones row
VMAX = 80.0                              # fit covers |v| in [0, VMAX]
L_MIN = float(np.log(0.01))
L_MAX = float(np.log(0.01 + VMAX))
Z_C = (L_MIN + L_MAX) / 2.0              # z = (L - Z_C) / Z_R
Z_R = (L_MAX - L_MIN) / 2.0
SMAX = 1.12                              # int8 full-scale
QSCALE = 127.0 / SMAX
DEQUANT = np.float32(SMAX / 127.0)
RING_STS = 8                             # output ring: 8 super-tiles
DVE_SHARE, ACT_SHARE = 7, 8              # cast split DVE:ACT (~0.467 DVE)

_NC_CACHE = None
LAST_RESULTS = None


def _build_bass():
    import concourse.bacc as bacc
    import concourse.mybir as mybir
    from concourse import tile
    from concourse.masks import make_identity

    f32 = mybir.dt.float32
    bf16 = mybir.dt.bfloat16
    i8 = mybir.dt.int8
    AF = mybir.ActivationFunctionType
    ALU = mybir.AluOpType

    nc = bacc.Bacc("TRN2")

    xv = nc.dram_tensor("xv", [128, TILES_PER_CORE], f32, kind="ExternalInput")
    wqb = nc.dram_tensor("wqb", [128, EMB], bf16, kind="ExternalInput")
    y = nc.dram_tensor("y", [N_PER_CORE, EMB], i8, kind="ExternalOutput")

    TOTAL_MM = N_SUPER * 8               # 512
    RING_COLS = RING_STS * SUPER * EMB   # 24576

    with tile.TileContext(nc) as tc:
        with (
            tc.tile_pool(name="consts", bufs=1) as consts,
            tc.tile_pool(name="work", bufs=1) as work,
            tc.tile_pool(name="lhsp", bufs=3) as lhsp,
            tc.tile_pool(name="ptr", bufs=1, space="PSUM") as ptr,
            tc.tile_pool(name="pout", bufs=3, space="PSUM") as pout,
        ):
            bias001 = consts.tile([128, 1], f32)
            nc.gpsimd.memset(bias001, 0.01)
            # Kick the ln table-set load while the x DMA is in flight.
            dummy = consts.tile([128, 1], f32)
            nc.scalar.activation(dummy, bias001, AF.Ln)

            ident = consts.tile([128, 128], f32)
            make_identity(nc, ident)
            ident_h = consts.tile([128, 128], bf16)
            nc.vector.tensor_copy(ident_h, ident)
            wqb_sb = consts.tile([128, EMB], bf16)
            nc.sync.dma_start(wqb_sb, wqb[:])

            x_sb = work.tile([128, TILES_PER_CORE], f32)
            nc.sync.dma_start(x_sb, xv[:])

            u = work.tile([128, TILES_PER_CORE], f32)
            lt = work.tile([128, TILES_PER_CORE], f32)
            z = work.tile([128, TILES_PER_CORE], bf16)
            z2 = work.tile([128, TILES_PER_CORE], bf16)
            nc.scalar.activation(u, x_sb, AF.Abs)
            nc.scalar.activation(lt, u, AF.Ln, bias=bias001[:, :])
            # z = (L - Z_C) / Z_R, one fused DVE tensor_scalar
            nc.vector.tensor_scalar(
                z, lt, 1.0 / Z_R, -Z_C / Z_R, ALU.mult, ALU.add
            )
            nc.vector.tensor_mul(z2, z, z)

            # P[p, st*128 + 32q + j] = basis_j(batch tile st*4+q):
            # j: 0=z 1=z^2 2=z^3 3=z^4 4=ones(bias row). Cols 5..31 of each
            # 32-slot group are never written nor contracted (K=5 slices).
            P = work.tile([128, N_SUPER * 128], bf16)
            P3 = P.rearrange("p (st q r) -> p st q r", q=SUPER, r=32)
            nc.vector.memset(P3[:, :, :, 4], 1.0)

            zv = z.rearrange("p (st q) -> p st q", q=SUPER)
            z2v = z2.rearrange("p (st q) -> p st q", q=SUPER)

            def emit_powers(lo, hi):
                ssl = slice(lo, hi)
                zc, z2c = zv[:, ssl, :], z2v[:, ssl, :]
                Pc = P3[:, ssl, :, :]
                nc.vector.tensor_copy(Pc[:, :, :, 0], zc)
                nc.vector.tensor_copy(Pc[:, :, :, 1], z2c)
                nc.vector.tensor_mul(Pc[:, :, :, 2], z2c, zc)
                nc.vector.tensor_mul(Pc[:, :, :, 3], z2c, z2c)

            HEAD_ST = 8   # first transpose batch (one packed ptr tile)
            emit_powers(0, HEAD_ST)

            yv = y.rearrange("(p t) e -> p t e", p=128)
            ring = work.tile([128, RING_COLS], i8)

            def emit_dma(s0, h):
                # Ring is in MM-issue order (col = 2048h + 512q + e within
                # the ST block); one store per (st, h) half keeps both the
                # cast dsts and the DMA src contiguous.  dst is a 3-dim AP
                # (q stride 1024, 512B contiguous runs).  Stores alternate
                # between the two HWDGE rings (sync / scalar sequencers).
                rbase = (s0 % RING_STS) * SUPER * EMB + 2048 * h
                src = ring[:, rbase : rbase + 2048].rearrange(
                    "p (q e) -> p q e", q=SUPER
                )
                dst = yv[:, s0 * SUPER : (s0 + 1) * SUPER, 512 * h : 512 * (h + 1)]
                eng = nc.sync if h == 0 else nc.scalar
                eng.dma_start(dst, src)

            # Transposes batched: ONE packed ptr tile holds EIGHT [128,128]
            # PE transposes (8 STs) in one PSUM bank, one DVE copy -> SBUF.
            # Batching bounds the PE tiling-mode switches (the 128x128
            # transpose vs 32-row-tiled MMs force a drain each way) and a
            # single ptr bank leaves FOUR pout bufs so casts stay fed
            # across the transpose phase.
            GROUP = 8                                  # STs per batch
            mm = 0
            ncast = 0
            st0 = 0
            for batch in range(N_SUPER // GROUP):
                if st0 == HEAD_ST:
                    emit_powers(HEAD_ST, N_SUPER)
                ptile = ptr.tile([128, 1024], bf16)
                for k in range(GROUP):
                    st = st0 + k
                    nc.tensor.transpose(
                        ptile[:, 128 * k : 128 * (k + 1)],
                        P[:, st * 128 : (st + 1) * 128],
                        ident_h,
                    )
                lhs_big = lhsp.tile([128, 1024], bf16)
                nc.vector.tensor_copy(lhs_big, ptile)

                if True:
                    for k in range(GROUP):
                        st = st0 + k
                        rbase = (st % RING_STS) * SUPER * EMB
                        # h-outer, q rotating through all four PE row groups:
                        # the LDWEIGHTS lookahead always has a free group, so
                        # adjacent MMs stream 2-way (~290ns/MM vs 354 when
                        # only two groups alternate).  Each pout tile is
                        # filled by two CONSECUTIVE MMs (q-pair of one h) and
                        # cast FD=1024 to the issue-order ring, contiguous.
                        for h in range(2):
                            for qp in (0, 2):
                                wt = pout.tile([128, 1024], f32, name="wt")
                                for q in (qp, qp + 1):
                                    nc.tensor.matmul(
                                        wt[:, 512 * (q - qp) : 512 * (q - qp + 1)],
                                        lhsT=lhs_big[
                                            32 * q : 32 * q + KDIM,
                                            128 * k : 128 * (k + 1),
                                        ],
                                        rhs=wqb_sb[
                                            32 * q : 32 * q + KDIM,
                                            512 * h : 512 * (h + 1),
                                        ],
                                        start=True,
                                        stop=True,
                                        tile_position=(32 * q, 0),
                                    )
                                    mm += 1
                                col = rbase + 2048 * h + 512 * qp
                                dst = ring[:, col : col + 1024]
                                sel = (ncast * DVE_SHARE) % (DVE_SHARE + ACT_SHARE)
                                if sel < DVE_SHARE:
                                    nc.vector.tensor_copy(dst, wt)
                                else:
                                    nc.scalar.copy(dst, wt)
                                ncast += 1
                            emit_dma(st, h)
                st0 += GROUP
            assert mm == TOTAL_MM, mm

    nc.finalize()
    return nc


def _get_nc():
    global _NC_CACHE
    if _NC_CACHE is None:
        _NC_CACHE = _build_bass()
    return _NC_CACHE


def _fit_coeffs(Q, W, b):
    """Chebyshev-node quartic fit of g(z) = polar(theta(z)) @ (W@Q)^T + b."""
    n_fit = 2001
    zf = np.cos(np.pi * (np.arange(n_fit) + 0.5) / n_fit)
    Lf = Z_C + Z_R * zf
    vf = np.exp(Lf) - 0.01
    theta = Lf * (np.pi / 85.0)
    s, c = np.sin(theta), np.cos(theta)
    dims = np.arange(1, D + 1)
    powers = np.where(dims < D, dims - 1, D)
    factor = np.where(dims < D, c[:, None], np.ones((n_fit, 1)))
    polar = (s[:, None] ** powers) * factor              # [n, D]
    wq = W.astype(np.float64) @ Q.astype(np.float64)     # [EMB, D]
    g = polar @ wq.T + b.astype(np.float64)[None, :]     # [n, EMB]
    V = np.vander(zf, KDIM, increasing=True)             # [n, 5] 1,z,..,z^4
    A, *_ = np.linalg.lstsq(V, g, rcond=None)            # [5, EMB]
    return A


def kernel(batch_val, Q, W, b):
    global LAST_RESULTS
    import ml_dtypes
    from concourse.bass_utils import run_bass_kernel_spmd

    batch_val = np.asarray(batch_val, dtype=np.float32)
    Q = np.asarray(Q, dtype=np.float32)
    W = np.asarray(W, dtype=np.float32)
    b = np.asarray(b, dtype=np.float32)

    A = _fit_coeffs(Q, W, b) * QSCALE                    # [5, EMB]
    # Device basis rows per 32-row group: j=0..3 -> z..z^4, j=4 -> ones(A0)
    wrows = np.concatenate([A[1:], A[:1]], axis=0)       # [5, EMB]
    wqb = np.zeros((128, EMB), dtype=ml_dtypes.bfloat16)
    for g in range(4):
        wqb[32 * g : 32 * g + KDIM, :] = wrows.astype(ml_dtypes.bfloat16)

    in_maps = []
    for core in range(N_CORES):
        sl = batch_val[core * N_PER_CORE : (core + 1) * N_PER_CORE]
        xc = sl.reshape(128, TILES_PER_CORE)
        in_maps.append({"xv": xc, "wqb": wqb})

    nc = _get_nc()
    LAST_RESULTS = run_bass_kernel_spmd(nc, in_maps, core_ids=list(range(N_CORES)))
    out = np.concatenate([r["y"] for r in LAST_RESULTS.results], axis=0)
    return out.astype(np.float32) * DEQUANT


# revision 49
# speedup vs baseline: 1.0036x; 1.0036x over previous
"""DiceEmbedding kernel for 8 Trainium2 NeuronCores.

Reference math (per element v of batch_val [262144]):
    theta    = ln(0.01 + |v|) / 85 * pi
    s, c     = sin(theta), cos(theta)
    polar    = [c, s*c, s^2*c, ..., s^8*c, s^10]           # [10]
    out      = (polar @ Q.T) @ W.T + b                     # [1024]

Key observation: out is a smooth 1-D function of L = ln(0.01+|v|) alone
(theta spans only [-0.17, 0.15] rad).  Host fits a QUARTIC in
z = (L - c)/r (z in [-1,1] for |v| <= 80) to the exact function
g(v) = polar @ (W@Q)^T + b per output column:  out ~= sum_k A_k z^k.
Fit residual < 1e-5 relative; total device error (bf16 powers, bf16 A,
int8 output) measures ~0.9% of absmax vs the 2% gate.

Device dataflow per core (data-parallel over N: 32768 elems per core):
  - batch slice arrives [128, 256] partition-major (x[p,t] = v[p*256+t])
  - ACT: |x| -> ln -> L (f32); DVE: z = (L-c)/r (bf16), z2 = z*z
  - DVE writes the 5-row basis [z, z2, z3, z4, ones] into P[128, 64*128]
    bf16 at cols st*128 + 32q + j (j<5; col 32q+4 is the memset ones row
    carrying A0; cols 5..31 untouched garbage, never contracted)
  - lhsT tiles via batched PE transposes: each packed [128,512] bf16
    PSUM tile holds FOUR [128,128] transposes (4 STs) in one bank, one
    DVE copy -> SBUF; batching 4-8 STs bounds the PE tiling-mode
    switches (128x128 transpose vs 32-row-tiled MMs drain the PE).
    (DMA-xbar transposes would free the PE entirely but hard-wedge the
    device next to the store DMAs — NRT_EXEC_UNIT_UNRECOVERABLE.)
  - per super-tile: 8 bf16 matmuls [K=5, N=512], tile_position=(32q,0),
    issued (q0h0, q1h0, q0h1, q1h1) so consecutive MMs alternate PE row
    groups (2-way concurrent streaming, hidden LDWEIGHTS) while each
    [128,1024] PSUM tile (2 banks; 3-buf pool + 2 transpose banks = all
    8 banks) fills its h0 then h1 chunk of one q
  - PSUM->SBUF f32->int8 casts at FD=1024, contiguous both sides (ring
    is in DRAM order; chunks (q,h0),(q,h1) are adjacent), split DVE:ACT
    ~ 10:11 (both engines read f32 PSUM at 1 elem/lane/cyc; FD=1024
    amortizes the ~120/172-cycle startup; strided dsts cost +~200cyc so
    contiguity matters)
  - per-ST 512 KiB contiguous DMA stores from an 8-ST int8 ring; host
    dequantizes (astype(f32) * SMAX/127)

Bottleneck: PSUM evacuation (every output crosses PSUM->SBUF on DVE or
ACT at 1 col/cycle) balanced against the PE stream (~290ns per 512-col
MM incl overheads).  Floor ~ 150 us busy + ~15 us framework pre/post-
amble; v1 (11-row basis, 512-col casts, per-pair PE transposes) ran
281 us.
"""

import numpy as np

D = 10
EMB = 1024
N_TOTAL = 262144
N_CORES = 8
N_PER_CORE = N_TOTAL // N_CORES          # 32768
TILES_PER_CORE = N_PER_CORE // 128       # 256
SUPER = 4                                # batch tiles per super-tile
N_SUPER = TILES_PER_CORE // SUPER        # 64
KDIM = 5                                 # quartic basis: z..z^4 + ones row
VMAX = 80.0                              # fit covers |v| in [0, VMAX]
L_MIN = float(np.log(0.01))
L_MAX = float(np.log(0.01 + VMAX))
Z_C = (L_MIN + L_MAX) / 2.0              # z = (L - Z_C) / Z_R
Z_R = (L_MAX - L_MIN) / 2.0
SMAX = 1.12                              # int8 full-scale
QSCALE = 127.0 / SMAX
DEQUANT = np.float32(SMAX / 127.0)
RING_STS = 8                             # output ring: 8 super-tiles
DVE_SHARE, ACT_SHARE = 7, 8              # cast split DVE:ACT (~0.467 DVE)

_NC_CACHE = None
LAST_RESULTS = None


def _build_bass():
    import concourse.bacc as bacc
    import concourse.mybir as mybir
    from concourse import tile
    from concourse.masks import make_identity

    f32 = mybir.dt.float32
    bf16 = mybir.dt.bfloat16
    i8 = mybir.dt.int8
    AF = mybir.ActivationFunctionType
    ALU = mybir.AluOpType

    nc = bacc.Bacc("TRN2")

    xv = nc.dram_tensor("xv", [128, TILES_PER_CORE], f32, kind="ExternalInput")
    wqb = nc.dram_tensor("wqb", [128, EMB], bf16, kind="ExternalInput")
    # Output split by EMB half: y[h, n, e'] = out[n, 512h + e'].  Both DMA
    # sides stay contiguous (2 KiB lines); the host interleaves the halves
    # during the dequant pass at no extra cost.
    y = nc.dram_tensor("y", [2, N_PER_CORE, EMB // 2], i8, kind="ExternalOutput")

    TOTAL_MM = N_SUPER * 8               # 512
    RING_COLS = RING_STS * SUPER * EMB   # 24576

    with tile.TileContext(nc) as tc:
        with (
            tc.tile_pool(name="consts", bufs=1) as consts,
            tc.tile_pool(name="work", bufs=1) as work,
            tc.tile_pool(name="lhsp", bufs=3) as lhsp,
            tc.tile_pool(name="ptr", bufs=1, space="PSUM") as ptr,
            tc.tile_pool(name="pout", bufs=3, space="PSUM") as pout,
        ):
            bias001 = consts.tile([128, 1], f32)
            nc.gpsimd.memset(bias001, 0.01)
            # Kick the ln table-set load while the x DMA is in flight.
            dummy = consts.tile([128, 1], f32)
            nc.scalar.activation(dummy, bias001, AF.Ln)

            ident = consts.tile([128, 128], f32)
            make_identity(nc, ident)
            ident_h = consts.tile([128, 128], bf16)
            nc.vector.tensor_copy(ident_h, ident)
            wqb_sb = consts.tile([128, EMB], bf16)
            nc.sync.dma_start(wqb_sb, wqb[:])

            x_sb = work.tile([128, TILES_PER_CORE], f32)
            nc.sync.dma_start(x_sb, xv[:])

            u = work.tile([128, TILES_PER_CORE], f32)
            lt = work.tile([128, TILES_PER_CORE], f32)
            z = work.tile([128, TILES_PER_CORE], bf16)
            z2 = work.tile([128, TILES_PER_CORE], bf16)
            nc.scalar.activation(u, x_sb, AF.Abs)
            nc.scalar.activation(lt, u, AF.Ln, bias=bias001[:, :])
            # z = (L - Z_C) / Z_R, one fused DVE tensor_scalar
            nc.vector.tensor_scalar(
                z, lt, 1.0 / Z_R, -Z_C / Z_R, ALU.mult, ALU.add
            )
            nc.vector.tensor_mul(z2, z, z)

            # P[p, st*128 + 32q + j] = basis_j(batch tile st*4+q):
            # j: 0=z 1=z^2 2=z^3 3=z^4 4=ones(bias row). Cols 5..31 of each
            # 32-slot group are never written nor contracted (K=5 slices).
            P = work.tile([128, N_SUPER * 128], bf16)
            P3 = P.rearrange("p (st q r) -> p st q r", q=SUPER, r=32)
            nc.vector.memset(P3[:, :, :, 4], 1.0)

            zv = z.rearrange("p (st q) -> p st q", q=SUPER)
            z2v = z2.rearrange("p (st q) -> p st q", q=SUPER)

            def emit_powers(lo, hi):
                ssl = slice(lo, hi)
                zc, z2c = zv[:, ssl, :], z2v[:, ssl, :]
                Pc = P3[:, ssl, :, :]
                nc.vector.tensor_copy(Pc[:, :, :, 0], zc)
                nc.vector.tensor_copy(Pc[:, :, :, 1], z2c)
                nc.vector.tensor_mul(Pc[:, :, :, 2], z2c, zc)
                nc.vector.tensor_mul(Pc[:, :, :, 3], z2c, z2c)

            HEAD_ST = 4   # first transpose batch (one packed ptr tile)
            emit_powers(0, HEAD_ST)

            yv = y.rearrange("h (p t) e -> h p t e", p=128)
            ring = work.tile([128, RING_COLS], i8)

            def emit_dma(s0, h):
                # Ring is in MM-issue order (col = 2048h + 512q + e within
                # the ST block); one store per (st, h) half keeps the cast
                # dsts, the DMA src AND the DMA dst contiguous (y is split
                # by half).  Stores alternate between the two HWDGE rings
                # (sync / scalar sequencers).
                rbase = (s0 % RING_STS) * SUPER * EMB + 2048 * h
                src = ring[:, rbase : rbase + 2048]
                dst = yv[h, :, s0 * SUPER : (s0 + 1) * SUPER, :]
                eng = nc.sync if h == 0 else nc.scalar
                eng.dma_start(dst, src.rearrange("p (q e) -> p q e", q=SUPER))

            # Transposes batched: ONE packed ptr tile holds EIGHT [128,128]
            # PE transposes (8 STs) in one PSUM bank, one DVE copy -> SBUF.
            # Batching bounds the PE tiling-mode switches (the 128x128
            # transpose vs 32-row-tiled MMs force a drain each way) and a
            # single ptr bank leaves FOUR pout bufs so casts stay fed
            # across the transpose phase.
            GROUP = 4                                  # STs per batch
            mm = 0
            ncast = 0
            st0 = 0
            for batch in range(N_SUPER // GROUP):
                if st0 == HEAD_ST:
                    emit_powers(HEAD_ST, N_SUPER)
                ptile = ptr.tile([128, 128 * GROUP], bf16)
                for k in range(GROUP):
                    st = st0 + k
                    nc.tensor.transpose(
                        ptile[:, 128 * k : 128 * (k + 1)],
                        P[:, st * 128 : (st + 1) * 128],
                        ident_h,
                    )
                lhs_big = lhsp.tile([128, 128 * GROUP], bf16)
                nc.vector.tensor_copy(lhs_big, ptile)

                if True:
                    for k in range(GROUP):
                        st = st0 + k
                        rbase = (st % RING_STS) * SUPER * EMB
                        # h-outer, q rotating through all four PE row groups:
                        # the LDWEIGHTS lookahead always has a free group, so
                        # adjacent MMs stream 2-way (~290ns/MM vs 354 when
                        # only two groups alternate).  Each pout tile is
                        # filled by two CONSECUTIVE MMs (q-pair of one h) and
                        # cast FD=1024 to the issue-order ring, contiguous.
                        for h in range(2):
                            for qp in (0, 2):
                                wt = pout.tile([128, 1024], f32, name="wt")
                                for q in (qp, qp + 1):
                                    nc.tensor.matmul(
                                        wt[:, 512 * (q - qp) : 512 * (q - qp + 1)],
                                        lhsT=lhs_big[
                                            32 * q : 32 * q + KDIM,
                                            128 * k : 128 * (k + 1),
                                        ],
                                        rhs=wqb_sb[
                                            32 * q : 32 * q + KDIM,
                                            512 * h : 512 * (h + 1),
                                        ],
                                        start=True,
                                        stop=True,
                                        tile_position=(32 * q, 0),
                                    )
                                    mm += 1
                                col = rbase + 2048 * h + 512 * qp
                                dst = ring[:, col : col + 1024]
                                sel = (ncast * DVE_SHARE) % (DVE_SHARE + ACT_SHARE)
                                if sel < DVE_SHARE:
                                    nc.vector.tensor_copy(dst, wt)
                                else:
                                    nc.scalar.copy(dst, wt)
                                ncast += 1
                            emit_dma(st, h)
                st0 += GROUP
            assert mm == TOTAL_MM, mm

    nc.finalize()
    return nc


def _get_nc():
    global _NC_CACHE
    if _NC_CACHE is None:
        _NC_CACHE = _build_bass()
    return _NC_CACHE


def _fit_coeffs(Q, W, b):
    """Chebyshev-node quartic fit of g(z) = polar(theta(z)) @ (W@Q)^T + b."""
    n_fit = 2001
    zf = np.cos(np.pi * (np.arange(n_fit) + 0.5) / n_fit)
    Lf = Z_C + Z_R * zf
    vf = np.exp(Lf) - 0.01
    theta = Lf * (np.pi / 85.0)
    s, c = np.sin(theta), np.cos(theta)
    dims = np.arange(1, D + 1)
    powers = np.where(dims < D, dims - 1, D)
    factor = np.where(dims < D, c[:, None], np.ones((n_fit, 1)))
    polar = (s[:, None] ** powers) * factor              # [n, D]
    wq = W.astype(np.float64) @ Q.astype(np.float64)     # [EMB, D]
    g = polar @ wq.T + b.astype(np.float64)[None, :]     # [n, EMB]
    V = np.vander(zf, KDIM, increasing=True)             # [n, 5] 1,z,..,z^4
    A, *_ = np.linalg.lstsq(V, g, rcond=None)            # [5, EMB]
    return A


def kernel(batch_val, Q, W, b):
    global LAST_RESULTS
    import ml_dtypes
    from concourse.bass_utils import run_bass_kernel_spmd

    batch_val = np.asarray(batch_val, dtype=np.float32)
    Q = np.asarray(Q, dtype=np.float32)
    W = np.asarray(W, dtype=np.float32)
    b = np.asarray(b, dtype=np.float32)

    A = _fit_coeffs(Q, W, b) * QSCALE                    # [5, EMB]
    # Device basis rows per 32-row group: j=0..3 -> z..z^4, j=4 -> ones(A0)
    wrows = np.concatenate([A[1:], A[:1]], axis=0)       # [5, EMB]
    wqb = np.zeros((128, EMB), dtype=ml_dtypes.bfloat16)
    for g in range(4):
        wqb[32 * g : 32 * g + KDIM, :] = wrows.astype(ml_dtypes.bfloat16)

    in_maps = []
    for core in range(N_CORES):
        sl = batch_val[core * N_PER_CORE : (core + 1) * N_PER_CORE]
        xc = sl.reshape(128, TILES_PER_CORE)
        in_maps.append({"xv": xc, "wqb": wqb})

    nc = _get_nc()
    LAST_RESULTS = run_bass_kernel_spmd(nc, in_maps, core_ids=list(range(N_CORES)))
    out = np.empty((N_TOTAL, EMB), dtype=np.float32)
    for core, r in enumerate(LAST_RESULTS.results):
        sl = out[core * N_PER_CORE : (core + 1) * N_PER_CORE]
        yk = r["y"]                          # [2, N_PER_CORE, 512] int8
        sl[:, : EMB // 2] = yk[0].astype(np.float32)
        sl[:, EMB // 2 :] = yk[1].astype(np.float32)
    out *= DEQUANT
    return out
